# revision 31
# baseline (speedup 1.0000x reference)
"""Trainium2 Bass kernel for nn_BinaryLinear: out = sign(x @ sign(W).T + bias).

Strategy
--------
Data-parallel over the 8192-token dim: each of the 8 cores gets 1024 tokens
and the full weight matrix.

On-chip compute (per core) is the NT GEMM z.T = sign(W) @ x.T on the
TensorEngine with the contraction (in_features) on the partition dim:

  psum[outf, tok] = sum_k w_s[k, outf] * x16[k, tok]

Precision: x is rounded to fp16 on the host and that is the ONLY precision
loss; sign(W) is exact (+-1) and PE accumulation is fp32. Host-side
simulation of the exact quantization error (acc_sim.py) shows 2195/33.5M
sign flips vs the fp32 reference => rel_err 0.0162 < the 2e-2 gate, HW
measures 2198. (The sim is trustworthy: it predicted the previous
fp16+fp8-residual scheme's flips exactly, 97 vs 97 measured.) This makes
the GEMM a pure-fp16 stream at 1 PE col/cycle -- 2048 matmuls x 512 tokens
per core -- vs 1.5 cyc/row for the fp16+fp8 scheme: 444us vs 691us of PE
time. A cheaper scheme does not exist on this PE: the only sub-1-cycle
mode is fp8 DoubleRow, and two fp8 terms carry ~8 effective mantissa bits
(rel_err ~0.028, fails), while fp16 carries 11.

sign(W) is computed on the host and shipped as fp8e4 (+-1 exact), already
packed in the SBUF layout the PE wants (per-partition-contiguous 8 KB
lines), so every DMA is full-line and no on-chip conversion is needed.
The matmul runs mixed fp8-stationary x fp16-moving at 1 col/cycle
(verified bit-identical to the host sim on HW). W traffic: 16 MB/core
instead of 64 MB fp32.

Schedule (from trace iteration):
- 14 warmup matmuls on a zeroed tile bridge the ~13us from engine preamble
  until the first x chunk lands, so the HAM clock gate is at 2.4 GHz when
  real matmuls start (cold PE runs at 1.2 GHz for the first ~5us of busy).
- x is resident (8 MB fp16), streamed as 16 x 512 KB chunks on the sync
  HWDGE queue in k order, just ahead of the PE's k-loop.
- W streams as 16 x 1 MB blocks on the gpsimd queue, triple-buffered; the
  first block is split in 4 so the k=0 LDWEIGHTS waits on only 256 KB, and
  blocks 1-2 are dep-gated on mb0 matmuls so the prefetch does not steal
  HBM bandwidth from the x stream (startup is HBM-bound: all 8 cores pull
  x simultaneously).
- Steady state measured 216.7 ns/matmul vs the 215.8 floor
  (512 cols / 2.4 GHz + NX); the PE never idles >0.5us after k=4.

The epilogue fuses bias-add + sign + PSUM->SBUF in one ScalarE activation
(bias is per-partition in the z.T layout), writing fp8 (+-1 exact), which
quarters the output DMA. Host converts back to fp32 and untransposes.
The final weight block runs its k-loops group-major so 3 of its 4
epilogues finish ~7-21us before the last matmul; the critical tail is one
activation + one same-engine (scalar-queue) DMA.

Measured: ~462-465us HW exec (vs 710-839us for the staged fp16+fp8
baseline); ~96% of the time is the PE matmul stream at its roofline.
Caveat: the chip occasionally sits in the P0 power state (PE capped at
2.0 GHz instead of 2.4), which inflates any kernel's time ~14%.
"""

import numpy as np

import concourse.tile as tile
import concourse.mybir as mybir
from concourse import bacc
from concourse.bass_utils import run_bass_kernel_spmd
from concourse.tile_rust import add_dep_helper

N_CORES = 8
N_TOK = 8192
D_IN = 4096
D_OUT = 4096
P = 128
T = N_TOK // N_CORES  # 1024 tokens per core
KT = D_IN // P  # 32 contraction tiles
MT = D_OUT // P  # 32 out-feature tiles
M2 = 2  # m-tiles per W block (256 outf cols)
MB = MT // M2  # 16 W blocks
TB = 512  # token block (one PSUM bank of fp32)
NB = T // TB  # 2 token blocks per core
XG = 2  # k-tiles per x DMA chunk
NXG = KT // XG  # 16 x DMA chunks

F32 = mybir.dt.float32
FP16 = mybir.dt.float16
FP8 = mybir.dt.float8e4
SIGN = mybir.ActivationFunctionType.Sign
E4M3 = mybir.dt.np(FP8)

_nc_cache = None


def build():
    """Build + compile the per-core Bass/Tile module (SPMD: same on all cores)."""
    global _nc_cache
    if _nc_cache is not None:
        return _nc_cache
    nc = bacc.Bacc("TRN2", target_bir_lowering=False, debug=False, num_devices=N_CORES)
    x_d = nc.dram_tensor("x16", [P, KT * T], FP16, kind="ExternalInput").ap()
    w_d = nc.dram_tensor("w8", [MB * P, KT * M2 * P], FP8, kind="ExternalInput").ap()
    b_d = nc.dram_tensor("bias_pk", [P, MT], F32, kind="ExternalInput").ap()
    out_d = nc.dram_tensor("out8_t", [D_OUT, T], FP8, kind="ExternalOutput").ap()

    with tile.TileContext(nc) as tc:
        with (
            tc.tile_pool(name="x", bufs=1) as x_pool,
            tc.tile_pool(name="wsb", bufs=3) as w_pool,
            tc.tile_pool(name="bias", bufs=1) as b_pool,
            tc.tile_pool(name="out", bufs=6) as out_pool,
            tc.tile_pool(name="warm", bufs=1) as warm_pool,
            tc.tile_pool(name="psum", bufs=8, space="PSUM") as psum_pool,
        ):
            # PE warmup: the HAM clock gate holds the PE at 1.2 GHz until it
            # has been busy ~3.4us. Run dummy matmuls on a zeroed tile while
            # the first x/W DMAs are in flight so the real matmuls start at
            # 2.4 GHz.
            # ~5.5us of dummy busy (what HAM needs to un-throttle), with a
            # fine-grained N=128 tail so the first real matmul waits at
            # most ~107ns for the dummy queue to drain once its x lands.
            warm = warm_pool.tile([P, TB], FP16, tag="warm")
            nc.gpsimd.memset(warm[:], 0.0)
            wps = psum_pool.tile([P, TB], F32, tag="psum", name="warm_ps")
            for i in range(10):
                nc.tensor.matmul(wps[:], warm[:, :P], warm[:], start=True,
                                 stop=True)
            for i in range(12):
                nc.tensor.matmul(wps[:, :P], warm[:, :P], warm[:, :P],
                                 start=True, stop=True)

            # DMA-path warmup: the first transfer of the run measures only
            # ~45 GB/s effective (cold SDMA/HBM path); a tiny no-consumer
            # DMA issued first absorbs the one-time cost so the real x
            # stream starts on a warm path.
            dwarm = warm_pool.tile([P, 32], FP16, tag="dwarm")
            nc.sync.dma_start(dwarm[:], x_d[:, 0:32])

            # bias, outf-partition-major: bias_sb[p, mo] = bias[mo*128 + p].
            # On the scalar queue so its issue slot does not delay the x
            # stream's head on sync (bias isn't needed until the first
            # activation, ~40us in).
            bias_sb = b_pool.tile([P, MT], F32, tag="bias")
            nc.scalar.dma_start(bias_sb[:], b_d[:, :])

            # Resident x, 16 chunks of 2 k-tiles (512 KB each), in k order
            # on one HWDGE queue (FIFO) so chunk g lands just ahead of the
            # PE's k=2g matmuls.
            # All x chunks on ONE HWDGE ring, in k order: a single FIFO ring
            # delivers chunks in exactly the order the PE consumes them.
            # (Splitting x across the SP+ACT rings for a bigger round-robin
            # share was tried and measured WORSE: bandwidth then splits
            # between the currently-needed chunk and later ones.)
            xt = []
            for g in range(NXG):
                th = x_pool.tile([P, XG, T], FP16, tag=f"xh_{g}", name=f"xh_{g}")
                if g == 0:
                    # Chunk 0 in two 256 KB pieces: the first real matmul
                    # gates on half the transfer.
                    nc.sync.dma_start(th[:, 0, :], x_d[:, 0:T])
                    nc.sync.dma_start(th[:, 1, :], x_d[:, T : 2 * T])
                else:
                    nc.sync.dma_start(th[:], x_d[:, g * XG * T : (g + 1) * XG * T])
                xt.append(th)

            def x_sl(k, n):
                return xt[k // XG][:, k % XG, n * TB : (n + 1) * TB]

            # mm_gates[(mb, k)]: a matmul to gate later W-block DMAs on, so
            # the W prefetch does not steal HBM bandwidth from the x stream
            # during the first block (x must fully land before mb=0 ends).
            # Notes from failed variants: an ungated W1 becomes
            # startup-critical (6.9us LDWEIGHTS stall on a slow-HBM run);
            # running mb0+mb1 as an 8-bank pair with both W blocks eager
            # does not absorb x jitter either (both blocks need the same
            # missing chunk, and the extra early W starves the x supply).
            gate_for_wdma = {1: (0, 24), 2: (1, 8)}
            mm_gates = {}
            for mb in range(MB):
                wsb = w_pool.tile([P, KT, M2 * P], FP8, tag="wsb",
                                  name=f"wsb_{mb}")
                if mb == 0:
                    # 4 pieces so the k=0 LDWEIGHTS only waits on a 256 KB
                    # transfer.
                    for q in range(4):
                        nc.gpsimd.dma_start(
                            wsb[:, q * 8 : (q + 1) * 8, :],
                            w_d[0:P, q * 8 * M2 * P : (q + 1) * 8 * M2 * P],
                        )
                else:
                    dma = nc.gpsimd.dma_start(wsb[:], w_d[mb * P : (mb + 1) * P, :])
                    if mb in gate_for_wdma:
                        add_dep_helper(dma.ins, mm_gates[gate_for_wdma[mb]],
                                       reason="delay W prefetch past x stream")

                psums = {
                    (mi, n): psum_pool.tile([P, TB], F32, tag="psum",
                                            name=f"ps_{mb}_{n}_{mi}")
                    for mi in range(M2)
                    for n in range(NB)
                }
                if mb == MB - 1:
                    # Group-major k-loops for the final block: each PSUM
                    # group stops ~7us apart, so 3 of the 4 epilogues
                    # finish well before the last matmul and the critical
                    # tail is a single activation + DMA, not a 4-deep
                    # serialized chain. (Not usable for mb=0: one group's
                    # full k-loop would outrun the x stream.)
                    loop = [
                        (mi, n, k)
                        for mi in range(M2)
                        for n in range(NB)
                        for k in range(KT)
                    ]
                else:
                    loop = [
                        (mi, n, k)
                        for k in range(KT)
                        for mi in range(M2)
                        for n in range(NB)
                    ]
                for mi, n, k in loop:
                    msl = slice(mi * P, (mi + 1) * P)
                    mm = nc.tensor.matmul(
                        psums[(mi, n)][:],
                        wsb[:, k, msl],
                        x_sl(k, n),
                        start=(k == 0),
                        stop=(k == KT - 1),
                    )
                    if mi == 0 and n == 0:
                        mm_gates[(mb, k)] = mm.ins
                for mi in range(M2):
                    m = mb * M2 + mi
                    for n in range(NB):
                        osb = out_pool.tile([P, TB], FP8, tag="osb",
                                            name=f"osb_{mb}_{n}_{mi}")
                        nc.scalar.activation(
                            osb[:], psums[(mi, n)][:], SIGN,
                            bias=bias_sb[:, m : m + 1],
                        )
                        # The very last output rides the scalar queue: it
                        # issues right after its activation on the same
                        # engine, skipping a cross-engine sem hop on the
                        # critical tail.
                        dma_eng = (
                            nc.scalar
                            if mb == MB - 1 and mi == M2 - 1 and n == NB - 1
                            else nc.sync
                        )
                        dma_eng.dma_start(
                            out_d[m * P : (m + 1) * P, n * TB : (n + 1) * TB],
                            osb[:],
                        )
    nc.compile()
    _nc_cache = nc
    return nc


def prep_in_maps(x, weight, bias):
    """Host-side layout prep: fp16 cast of x, sign(W)->fp8, packing."""
    x = np.asarray(x, dtype=np.float32)
    weight = np.asarray(weight, dtype=np.float32)
    bias = np.asarray(bias, dtype=np.float32)

    x16 = x.astype(np.float16)
    # w8[p, k, mi, c] = sign(W)[mb*256 + mi*128 + c, k*128 + p], per block mb
    S = np.sign(weight).astype(np.float32)
    w8 = (
        S.reshape(MB, M2, P, KT, P)
        .transpose(0, 4, 3, 1, 2)  # [mb, p, k, mi, c]
        .reshape(MB * P, KT * M2 * P)
    ).astype(E4M3)
    w8 = np.ascontiguousarray(w8)
    bias_pk = np.ascontiguousarray(bias.reshape(MT, P).T)

    in_maps = []
    for c in range(N_CORES):
        xc = x16[c * T : (c + 1) * T]  # [T, D_IN]
        # xp[p, k, t] = x16[c*T + t, k*128 + p]
        xp = np.ascontiguousarray(
            xc.reshape(T, KT, P).transpose(2, 1, 0).reshape(P, KT * T)
        )
        in_maps.append({"x16": xp, "w8": w8, "bias_pk": bias_pk})
    return in_maps


def run(x, weight, bias, **spmd_kwargs):
    """Run on the 8 cores; returns (full_output, BassKernelResults)."""
    nc = build()
    in_maps = prep_in_maps(x, weight, bias)
    res = run_bass_kernel_spmd(nc, in_maps, core_ids=list(range(N_CORES)), **spmd_kwargs)
    out = np.empty((N_TOK, D_OUT), dtype=np.float32)
    for c in range(N_CORES):
        out[c * T : (c + 1) * T, :] = res.results[c]["out8_t"].astype(np.float32).T
    return out, res


def kernel(x, weight, bias):
    out, _ = run(x, weight, bias)
    return out


# revision 33
# speedup vs baseline: 1.1874x; 1.1874x over previous
"""Trainium2 Bass kernel for nn_BinaryLinear: out = sign(x @ sign(W).T + bias).

Strategy
--------
Data-parallel over the 8192-token dim: each of the 8 cores gets 1024 tokens
and the full weight matrix.

On-chip compute (per core) is the NT GEMM z.T = sign(W) @ x.T on the
TensorEngine with the contraction (in_features) on the partition dim:

  psum[outf, tok] = sum_k w_s[k, outf] * x16[k, tok]

Precision: x is rounded to fp16 on the host and that is the ONLY precision
loss; sign(W) is exact (+-1) and PE accumulation is fp32. Host-side
simulation of the exact quantization error (acc_sim.py) shows 2195/33.5M
sign flips vs the fp32 reference => rel_err 0.0162 < the 2e-2 gate, HW
measures 2198. (The sim is trustworthy: it predicted the previous
fp16+fp8-residual scheme's flips exactly, 97 vs 97 measured.) This makes
the GEMM a pure-fp16 stream at 1 PE col/cycle -- 2048 matmuls x 512 tokens
per core -- vs 1.5 cyc/row for the fp16+fp8 scheme: 444us vs 691us of PE
time. A cheaper scheme does not exist on this PE: the only sub-1-cycle
mode is fp8 DoubleRow, and two fp8 terms carry ~8 effective mantissa bits
(rel_err ~0.028, fails), while fp16 carries 11.

sign(W) is computed on the host and shipped as fp8e4 (+-1 exact), already
packed in the SBUF layout the PE wants (per-partition-contiguous 8 KB
lines), so every DMA is full-line and no on-chip conversion is needed.
The matmul runs mixed fp8-stationary x fp16-moving at 1 col/cycle
(verified bit-identical to the host sim on HW). W traffic: 16 MB/core
instead of 64 MB fp32.

Schedule (from trace iteration):
- 14 warmup matmuls on a zeroed tile bridge the ~13us from engine preamble
  until the first x chunk lands, so the HAM clock gate is at 2.4 GHz when
  real matmuls start (cold PE runs at 1.2 GHz for the first ~5us of busy).
- x is resident (8 MB fp16), streamed as 16 x 512 KB chunks on the sync
  HWDGE queue in k order, just ahead of the PE's k-loop.
- W streams as 16 x 1 MB blocks on the gpsimd queue, triple-buffered; the
  first block is split in 4 so the k=0 LDWEIGHTS waits on only 256 KB, and
  blocks 1-2 are dep-gated on mb0 matmuls so the prefetch does not steal
  HBM bandwidth from the x stream (startup is HBM-bound: all 8 cores pull
  x simultaneously).
- Steady state measured 216.7 ns/matmul vs the 215.8 floor
  (512 cols / 2.4 GHz + NX); the PE never idles >0.5us after k=4.

The epilogue fuses bias-add + sign + PSUM->SBUF in one ScalarE activation
(bias is per-partition in the z.T layout), writing fp8 (+-1 exact), which
quarters the output DMA. Host converts back to fp32 and untransposes.
The final weight block runs its k-loops group-major so 3 of its 4
epilogues finish ~7-21us before the last matmul; the critical tail is one
activation + one same-engine (scalar-queue) DMA.

Measured: ~462-465us HW exec (vs 710-839us for the staged fp16+fp8
baseline); ~96% of the time is the PE matmul stream at its roofline.
Caveat: the chip occasionally sits in the P0 power state (PE capped at
2.0 GHz instead of 2.4), which inflates any kernel's time ~14%.
"""

import numpy as np

import concourse.tile as tile
import concourse.mybir as mybir
from concourse import bacc
from concourse.bass_utils import run_bass_kernel_spmd
from concourse.tile_rust import add_dep_helper

N_CORES = 8
N_TOK = 8192
D_IN = 4096
D_OUT = 4096
P = 128
T = N_TOK // N_CORES  # 1024 tokens per core
KT = D_IN // P  # 32 contraction tiles
MT = D_OUT // P  # 32 out-feature tiles
M2 = 2  # m-tiles per W block (256 outf cols)
MB = MT // M2  # 16 W blocks
TB = 512  # token block (one PSUM bank of fp32)
NB = T // TB  # 2 token blocks per core
XG = 2  # k-tiles per x DMA chunk
NXG = KT // XG  # 16 x DMA chunks

F32 = mybir.dt.float32
FP16 = mybir.dt.float16
FP8 = mybir.dt.float8e4
SIGN = mybir.ActivationFunctionType.Sign
E4M3 = mybir.dt.np(FP8)

_nc_cache = None


def build():
    """Build + compile the per-core Bass/Tile module (SPMD: same on all cores)."""
    global _nc_cache
    if _nc_cache is not None:
        return _nc_cache
    nc = bacc.Bacc("TRN2", target_bir_lowering=False, debug=False, num_devices=N_CORES)
    x_d = nc.dram_tensor("x16", [P, KT * T], FP16, kind="ExternalInput").ap()
    w_d = nc.dram_tensor("w8", [MB * P, KT * M2 * P], FP8, kind="ExternalInput").ap()
    b_d = nc.dram_tensor("bias_pk", [P, MT], F32, kind="ExternalInput").ap()
    out_d = nc.dram_tensor("out8_t", [D_OUT, T], FP8, kind="ExternalOutput").ap()

    with tile.TileContext(nc) as tc:
        with (
            tc.tile_pool(name="x", bufs=1) as x_pool,
            tc.tile_pool(name="wsb", bufs=3) as w_pool,
            tc.tile_pool(name="bias", bufs=1) as b_pool,
            tc.tile_pool(name="out", bufs=6) as out_pool,
            tc.tile_pool(name="warm", bufs=1) as warm_pool,
            tc.tile_pool(name="psum", bufs=8, space="PSUM") as psum_pool,
        ):
            # PE warmup: the HAM clock gate holds the PE at 1.2 GHz until it
            # has been busy ~3.4us. Run dummy matmuls on a zeroed tile while
            # the first x/W DMAs are in flight so the real matmuls start at
            # 2.4 GHz.
            # ~5.5us of dummy busy (what HAM needs to un-throttle), with a
            # fine-grained N=128 tail so the first real matmul waits at
            # most ~107ns for the dummy queue to drain once its x lands.
            warm = warm_pool.tile([P, TB], FP16, tag="warm")
            nc.gpsimd.memset(warm[:], 0.0)
            wps = psum_pool.tile([P, TB], F32, tag="psum", name="warm_ps")
            for i in range(10):
                nc.tensor.matmul(wps[:], warm[:, :P], warm[:], start=True,
                                 stop=True)
            for i in range(12):
                nc.tensor.matmul(wps[:, :P], warm[:, :P], warm[:, :P],
                                 start=True, stop=True)

            # DMA-path warmup: the run's first transfer measures only
            # ~45 GB/s effective (cold SDMA/HBM path); a tiny no-consumer
            # DMA issued first absorbs the one-time cost so the real x
            # stream starts on a warm path.
            dwarm = warm_pool.tile([P, 32], FP16, tag="dwarm")
            nc.sync.dma_start(dwarm[:], x_d[:, 0:32])

            # bias, outf-partition-major: bias_sb[p, mo] = bias[mo*128 + p].
            # On the scalar queue so its issue slot does not delay the x
            # stream's head on sync (bias isn't needed until the first
            # activation, ~40us in).
            bias_sb = b_pool.tile([P, MT], F32, tag="bias")
            nc.scalar.dma_start(bias_sb[:], b_d[:, :])

            # Resident x, 16 chunks of 2 k-tiles (512 KB each), in k order
            # on one HWDGE queue (FIFO) so chunk g lands just ahead of the
            # PE's k=2g matmuls.
            # All x chunks on ONE HWDGE ring, in k order: a single FIFO ring
            # delivers chunks in exactly the order the PE consumes them.
            # (Splitting x across the SP+ACT rings for a bigger round-robin
            # share was tried and measured WORSE: bandwidth then splits
            # between the currently-needed chunk and later ones.)
            xt = []
            for g in range(NXG):
                th = x_pool.tile([P, XG, T], FP16, tag=f"xh_{g}", name=f"xh_{g}")
                if g == 0:
                    # Chunk 0 in two 256 KB pieces: the first real matmul
                    # gates on half the transfer.
                    nc.sync.dma_start(th[:, 0, :], x_d[:, 0:T])
                    nc.sync.dma_start(th[:, 1, :], x_d[:, T : 2 * T])
                else:
                    nc.sync.dma_start(th[:], x_d[:, g * XG * T : (g + 1) * XG * T])
                xt.append(th)

            def x_sl(k, n):
                return xt[k // XG][:, k % XG, n * TB : (n + 1) * TB]

            # mm_gates[(mb, k)]: a matmul to gate later W-block DMAs on, so
            # the W prefetch does not steal HBM bandwidth from the x stream
            # during the first block (x must fully land before mb=0 ends).
            # Notes from failed variants: an ungated W1 becomes
            # startup-critical (6.9us LDWEIGHTS stall on a slow-HBM run);
            # running mb0+mb1 as an 8-bank pair with both W blocks eager
            # does not absorb x jitter either (both blocks need the same
            # missing chunk, and the extra early W starves the x supply).
            gate_for_wdma = {1: (0, 24), 2: (1, 8)}
            mm_gates = {}
            for mb in range(MB):
                wsb = w_pool.tile([P, KT, M2 * P], FP8, tag="wsb",
                                  name=f"wsb_{mb}")
                if mb == 0:
                    # 4 pieces so the k=0 LDWEIGHTS only waits on a 256 KB
                    # transfer.
                    for q in range(4):
                        nc.gpsimd.dma_start(
                            wsb[:, q * 8 : (q + 1) * 8, :],
                            w_d[0:P, q * 8 * M2 * P : (q + 1) * 8 * M2 * P],
                        )
                else:
                    dma = nc.gpsimd.dma_start(wsb[:], w_d[mb * P : (mb + 1) * P, :])
                    if mb in gate_for_wdma:
                        add_dep_helper(dma.ins, mm_gates[gate_for_wdma[mb]],
                                       reason="delay W prefetch past x stream")

                psums = {
                    (mi, n): psum_pool.tile([P, TB], F32, tag="psum",
                                            name=f"ps_{mb}_{n}_{mi}")
                    for mi in range(M2)
                    for n in range(NB)
                }
                if mb == MB - 1:
                    # Group-major k-loops for the final block: each PSUM
                    # group stops ~7us apart, so 3 of the 4 epilogues
                    # finish well before the last matmul and the critical
                    # tail is a single activation + DMA, not a 4-deep
                    # serialized chain. (Not usable for mb=0: one group's
                    # full k-loop would outrun the x stream.)
                    loop = [
                        (mi, n, k)
                        for mi in range(M2)
                        for n in range(NB)
                        for k in range(KT)
                    ]
                else:
                    loop = [
                        (mi, n, k)
                        for k in range(KT)
                        for mi in range(M2)
                        for n in range(NB)
                    ]
                for mi, n, k in loop:
                    msl = slice(mi * P, (mi + 1) * P)
                    mm = nc.tensor.matmul(
                        psums[(mi, n)][:],
                        wsb[:, k, msl],
                        x_sl(k, n),
                        start=(k == 0),
                        stop=(k == KT - 1),
                    )
                    if mi == 0 and n == 0:
                        mm_gates[(mb, k)] = mm.ins
                for mi in range(M2):
                    m = mb * M2 + mi
                    for n in range(NB):
                        osb = out_pool.tile([P, TB], FP8, tag="osb",
                                            name=f"osb_{mb}_{n}_{mi}")
                        nc.scalar.activation(
                            osb[:], psums[(mi, n)][:], SIGN,
                            bias=bias_sb[:, m : m + 1],
                        )
                        # The very last output rides the scalar queue: it
                        # issues right after its activation on the same
                        # engine, skipping a cross-engine sem hop on the
                        # critical tail.
                        dma_eng = (
                            nc.scalar
                            if mb == MB - 1 and mi == M2 - 1 and n == NB - 1
                            else nc.sync
                        )
                        dma_eng.dma_start(
                            out_d[m * P : (m + 1) * P, n * TB : (n + 1) * TB],
                            osb[:],
                        )
    nc.compile()
    _nc_cache = nc
    return nc


def prep_in_maps(x, weight, bias):
    """Host-side layout prep: fp16 cast of x, sign(W)->fp8, packing."""
    x = np.asarray(x, dtype=np.float32)
    weight = np.asarray(weight, dtype=np.float32)
    bias = np.asarray(bias, dtype=np.float32)

    x16 = x.astype(np.float16)
    # w8[p, k, mi, c] = sign(W)[mb*256 + mi*128 + c, k*128 + p], per block mb
    S = np.sign(weight).astype(np.float32)
    w8 = (
        S.reshape(MB, M2, P, KT, P)
        .transpose(0, 4, 3, 1, 2)  # [mb, p, k, mi, c]
        .reshape(MB * P, KT * M2 * P)
    ).astype(E4M3)
    w8 = np.ascontiguousarray(w8)
    bias_pk = np.ascontiguousarray(bias.reshape(MT, P).T)

    in_maps = []
    for c in range(N_CORES):
        xc = x16[c * T : (c + 1) * T]  # [T, D_IN]
        # xp[p, k, t] = x16[c*T + t, k*128 + p]
        xp = np.ascontiguousarray(
            xc.reshape(T, KT, P).transpose(2, 1, 0).reshape(P, KT * T)
        )
        in_maps.append({"x16": xp, "w8": w8, "bias_pk": bias_pk})
    return in_maps


def run(x, weight, bias, **spmd_kwargs):
    """Run on the 8 cores; returns (full_output, BassKernelResults)."""
    nc = build()
    in_maps = prep_in_maps(x, weight, bias)
    res = run_bass_kernel_spmd(nc, in_maps, core_ids=list(range(N_CORES)), **spmd_kwargs)
    out = np.empty((N_TOK, D_OUT), dtype=np.float32)
    for c in range(N_CORES):
        out[c * T : (c + 1) * T, :] = res.results[c]["out8_t"].astype(np.float32).T
    return out, res


def kernel(x, weight, bias):
    out, _ = run(x, weight, bias)
    return out


# revision 34
# speedup vs baseline: 1.1944x; 1.0059x over previous
"""Trainium2 Bass kernel for nn_BinaryLinear: out = sign(x @ sign(W).T + bias).

Strategy
--------
Data-parallel over the 8192-token dim: each of the 8 cores gets 1024 tokens
and the full weight matrix.

On-chip compute (per core) is the NT GEMM z.T = sign(W) @ x.T on the
TensorEngine with the contraction (in_features) on the partition dim:

  psum[outf, tok] = sum_k w_s[k, outf] * x16[k, tok]

Precision: x is rounded to fp16 on the host and that is the ONLY precision
loss; sign(W) is exact (+-1) and PE accumulation is fp32. Host-side
simulation of the exact quantization error (acc_sim.py) shows 2195/33.5M
sign flips vs the fp32 reference => rel_err 0.0162 < the 2e-2 gate, HW
measures 2198. (The sim is trustworthy: it predicted the previous
fp16+fp8-residual scheme's flips exactly, 97 vs 97 measured.) This makes
the GEMM a pure-fp16 stream at 1 PE col/cycle -- 2048 matmuls x 512 tokens
per core -- vs 1.5 cyc/row for the fp16+fp8 scheme: 444us vs 691us of PE
time. A cheaper scheme does not exist on this PE: the only sub-1-cycle
mode is fp8 DoubleRow, and two fp8 terms carry ~8 effective mantissa bits
(rel_err ~0.028, fails), while fp16 carries 11.

sign(W) is computed on the host and shipped as fp8e4 (+-1 exact), already
packed in the SBUF layout the PE wants (per-partition-contiguous 8 KB
lines), so every DMA is full-line and no on-chip conversion is needed.
The matmul runs mixed fp8-stationary x fp16-moving at 1 col/cycle
(verified bit-identical to the host sim on HW). W traffic: 16 MB/core
instead of 64 MB fp32.

Schedule (from trace iteration):
- 14 warmup matmuls on a zeroed tile bridge the ~13us from engine preamble
  until the first x chunk lands, so the HAM clock gate is at 2.4 GHz when
  real matmuls start (cold PE runs at 1.2 GHz for the first ~5us of busy).
- x is resident (8 MB fp16), streamed as 16 x 512 KB chunks on the sync
  HWDGE queue in k order, just ahead of the PE's k-loop.
- W streams as 16 x 1 MB blocks on the gpsimd queue, triple-buffered; the
  first block is split in 4 so the k=0 LDWEIGHTS waits on only 256 KB, and
  blocks 1-2 are dep-gated on mb0 matmuls so the prefetch does not steal
  HBM bandwidth from the x stream (startup is HBM-bound: all 8 cores pull
  x simultaneously).
- Steady state measured 216.7 ns/matmul vs the 215.8 floor
  (512 cols / 2.4 GHz + NX); the PE never idles >0.5us after k=4.

The epilogue fuses bias-add + sign + PSUM->SBUF in one ScalarE activation
(bias is per-partition in the z.T layout), writing fp8 (+-1 exact), which
quarters the output DMA. Host converts back to fp32 and untransposes.
The final weight block runs its k-loops group-major so 3 of its 4
epilogues finish ~7-21us before the last matmul; the critical tail is one
activation + one same-engine (scalar-queue) DMA.

Measured: ~462-465us HW exec (vs 710-839us for the staged fp16+fp8
baseline); ~96% of the time is the PE matmul stream at its roofline.
Caveat: the chip occasionally sits in the P0 power state (PE capped at
2.0 GHz instead of 2.4), which inflates any kernel's time ~14%.
"""

import numpy as np

import concourse.tile as tile
import concourse.mybir as mybir
from concourse import bacc
from concourse.bass_utils import run_bass_kernel_spmd
from concourse.tile_rust import add_dep_helper

N_CORES = 8
N_TOK = 8192
D_IN = 4096
D_OUT = 4096
P = 128
T = N_TOK // N_CORES  # 1024 tokens per core
KT = D_IN // P  # 32 contraction tiles
MT = D_OUT // P  # 32 out-feature tiles
M2 = 2  # m-tiles per W block (256 outf cols)
MB = MT // M2  # 16 W blocks
TB = 512  # token block (one PSUM bank of fp32)
NB = T // TB  # 2 token blocks per core
XG = 2  # k-tiles per x DMA chunk
NXG = KT // XG  # 16 x DMA chunks

F32 = mybir.dt.float32
FP16 = mybir.dt.float16
FP8 = mybir.dt.float8e4
SIGN = mybir.ActivationFunctionType.Sign
E4M3 = mybir.dt.np(FP8)

_nc_cache = None


def build():
    """Build + compile the per-core Bass/Tile module (SPMD: same on all cores)."""
    global _nc_cache
    if _nc_cache is not None:
        return _nc_cache
    nc = bacc.Bacc("TRN2", target_bir_lowering=False, debug=False, num_devices=N_CORES)
    x_d = nc.dram_tensor("x16", [P, KT * T], FP16, kind="ExternalInput").ap()
    w_d = nc.dram_tensor("w8", [MB * P, KT * M2 * P], FP8, kind="ExternalInput").ap()
    b_d = nc.dram_tensor("bias_pk", [P, MT], F32, kind="ExternalInput").ap()
    out_d = nc.dram_tensor("out8_t", [D_OUT, T], FP8, kind="ExternalOutput").ap()

    with tile.TileContext(nc) as tc:
        with (
            tc.tile_pool(name="x", bufs=1) as x_pool,
            tc.tile_pool(name="wsb", bufs=3) as w_pool,
            tc.tile_pool(name="bias", bufs=1) as b_pool,
            tc.tile_pool(name="out", bufs=6) as out_pool,
            tc.tile_pool(name="warm", bufs=1) as warm_pool,
            tc.tile_pool(name="psum", bufs=8, space="PSUM") as psum_pool,
        ):
            # PE warmup: the HAM clock gate holds the PE at 1.2 GHz until it
            # has been busy ~3.4us. Run dummy matmuls on a zeroed tile while
            # the first x/W DMAs are in flight so the real matmuls start at
            # 2.4 GHz.
            # ~5.5us of dummy busy (what HAM needs to un-throttle), with a
            # fine-grained N=128 tail so the first real matmul waits at
            # most ~107ns for the dummy queue to drain once its x lands.
            warm = warm_pool.tile([P, TB], FP16, tag="warm")
            nc.gpsimd.memset(warm[:], 0.0)
            wps = psum_pool.tile([P, TB], F32, tag="psum", name="warm_ps")
            for i in range(10):
                nc.tensor.matmul(wps[:], warm[:, :P], warm[:], start=True,
                                 stop=True)
            for i in range(12):
                nc.tensor.matmul(wps[:, :P], warm[:, :P], warm[:, :P],
                                 start=True, stop=True)

            # bias, outf-partition-major: bias_sb[p, mo] = bias[mo*128 + p].
            # (A tiny "DMA-path warmup" transfer ahead of the x stream was
            # tested twice and showed no benefit — the slow first transfer
            # is 8-core startup contention, not a one-time cold cost.)
            # On the scalar queue so its issue slot does not delay the x
            # stream's head on sync (bias isn't needed until the first
            # activation, ~40us in).
            bias_sb = b_pool.tile([P, MT], F32, tag="bias")
            nc.scalar.dma_start(bias_sb[:], b_d[:, :])

            # Resident x, 16 chunks of 2 k-tiles (512 KB each), in k order
            # on one HWDGE queue (FIFO) so chunk g lands just ahead of the
            # PE's k=2g matmuls.
            # All x chunks on ONE HWDGE ring, in k order: a single FIFO ring
            # delivers chunks in exactly the order the PE consumes them.
            # (Splitting x across the SP+ACT rings for a bigger round-robin
            # share was tried and measured WORSE: bandwidth then splits
            # between the currently-needed chunk and later ones.)
            xt = []
            for g in range(NXG):
                th = x_pool.tile([P, XG, T], FP16, tag=f"xh_{g}", name=f"xh_{g}")
                if g == 0:
                    # Chunk 0 in two 256 KB pieces: the first real matmul
                    # gates on half the transfer.
                    nc.sync.dma_start(th[:, 0, :], x_d[:, 0:T])
                    nc.sync.dma_start(th[:, 1, :], x_d[:, T : 2 * T])
                else:
                    nc.sync.dma_start(th[:], x_d[:, g * XG * T : (g + 1) * XG * T])
                xt.append(th)

            def x_sl(k, n):
                return xt[k // XG][:, k % XG, n * TB : (n + 1) * TB]

            # mm_gates[(mb, k)]: a matmul to gate later W-block DMAs on, so
            # the W prefetch does not steal HBM bandwidth from the x stream
            # during the first block (x must fully land before mb=0 ends).
            # Notes from failed variants: an ungated W1 becomes
            # startup-critical (6.9us LDWEIGHTS stall on a slow-HBM run);
            # running mb0+mb1 as an 8-bank pair with both W blocks eager
            # does not absorb x jitter either (both blocks need the same
            # missing chunk, and the extra early W starves the x supply).
            gate_for_wdma = {1: (0, 24), 2: (1, 8)}
            mm_gates = {}
            for mb in range(MB):
                wsb = w_pool.tile([P, KT, M2 * P], FP8, tag="wsb",
                                  name=f"wsb_{mb}")
                if mb == 0:
                    # 4 pieces so the k=0 LDWEIGHTS only waits on a 256 KB
                    # transfer.
                    for q in range(4):
                        nc.gpsimd.dma_start(
                            wsb[:, q * 8 : (q + 1) * 8, :],
                            w_d[0:P, q * 8 * M2 * P : (q + 1) * 8 * M2 * P],
                        )
                else:
                    dma = nc.gpsimd.dma_start(wsb[:], w_d[mb * P : (mb + 1) * P, :])
                    if mb in gate_for_wdma:
                        add_dep_helper(dma.ins, mm_gates[gate_for_wdma[mb]],
                                       reason="delay W prefetch past x stream")

                psums = {
                    (mi, n): psum_pool.tile([P, TB], F32, tag="psum",
                                            name=f"ps_{mb}_{n}_{mi}")
                    for mi in range(M2)
                    for n in range(NB)
                }
                if mb == MB - 1:
                    # Group-major k-loops for the final block: each PSUM
                    # group stops ~7us apart, so 3 of the 4 epilogues
                    # finish well before the last matmul and the critical
                    # tail is a single activation + DMA, not a 4-deep
                    # serialized chain. (Not usable for mb=0: one group's
                    # full k-loop would outrun the x stream.)
                    loop = [
                        (mi, n, k)
                        for mi in range(M2)
                        for n in range(NB)
                        for k in range(KT)
                    ]
                else:
                    loop = [
                        (mi, n, k)
                        for k in range(KT)
                        for mi in range(M2)
                        for n in range(NB)
                    ]
                for mi, n, k in loop:
                    msl = slice(mi * P, (mi + 1) * P)
                    mm = nc.tensor.matmul(
                        psums[(mi, n)][:],
                        wsb[:, k, msl],
                        x_sl(k, n),
                        start=(k == 0),
                        stop=(k == KT - 1),
                    )
                    if mi == 0 and n == 0:
                        mm_gates[(mb, k)] = mm.ins
                for mi in range(M2):
                    m = mb * M2 + mi
                    for n in range(NB):
                        osb = out_pool.tile([P, TB], FP8, tag="osb",
                                            name=f"osb_{mb}_{n}_{mi}")
                        nc.scalar.activation(
                            osb[:], psums[(mi, n)][:], SIGN,
                            bias=bias_sb[:, m : m + 1],
                        )
                        # The very last output rides the scalar queue: it
                        # issues right after its activation on the same
                        # engine, skipping a cross-engine sem hop on the
                        # critical tail.
                        dma_eng = (
                            nc.scalar
                            if mb == MB - 1 and mi == M2 - 1 and n == NB - 1
                            else nc.sync
                        )
                        dma_eng.dma_start(
                            out_d[m * P : (m + 1) * P, n * TB : (n + 1) * TB],
                            osb[:],
                        )
    nc.compile()
    _nc_cache = nc
    return nc


def prep_in_maps(x, weight, bias):
    """Host-side layout prep: fp16 cast of x, sign(W)->fp8, packing."""
    x = np.asarray(x, dtype=np.float32)
    weight = np.asarray(weight, dtype=np.float32)
    bias = np.asarray(bias, dtype=np.float32)

    x16 = x.astype(np.float16)
    # w8[p, k, mi, c] = sign(W)[mb*256 + mi*128 + c, k*128 + p], per block mb
    S = np.sign(weight).astype(np.float32)
    w8 = (
        S.reshape(MB, M2, P, KT, P)
        .transpose(0, 4, 3, 1, 2)  # [mb, p, k, mi, c]
        .reshape(MB * P, KT * M2 * P)
    ).astype(E4M3)
    w8 = np.ascontiguousarray(w8)
    bias_pk = np.ascontiguousarray(bias.reshape(MT, P).T)

    in_maps = []
    for c in range(N_CORES):
        xc = x16[c * T : (c + 1) * T]  # [T, D_IN]
        # xp[p, k, t] = x16[c*T + t, k*128 + p]
        xp = np.ascontiguousarray(
            xc.reshape(T, KT, P).transpose(2, 1, 0).reshape(P, KT * T)
        )
        in_maps.append({"x16": xp, "w8": w8, "bias_pk": bias_pk})
    return in_maps


def run(x, weight, bias, **spmd_kwargs):
    """Run on the 8 cores; returns (full_output, BassKernelResults)."""
    nc = build()
    in_maps = prep_in_maps(x, weight, bias)
    res = run_bass_kernel_spmd(nc, in_maps, core_ids=list(range(N_CORES)), **spmd_kwargs)
    out = np.empty((N_TOK, D_OUT), dtype=np.float32)
    for c in range(N_CORES):
        out[c * T : (c + 1) * T, :] = res.results[c]["out8_t"].astype(np.float32).T
    return out, res


def kernel(x, weight, bias):
    out, _ = run(x, weight, bias)
    return out
